# revision 74
# baseline (speedup 1.0000x reference)
"""Bass/Trainium2 kernel for DenseAtt: out = sigmoid(x@w_i [:,None] + x@w_j [None,:] + b).

Sharding: rows of the (8192, 8192) output are split across 8 NeuronCores
(1024 rows each). Each core receives the full x^T (bf16, host-pretransposed),
a packed constant block (identity, biases, weights, and its local x^T
columns), and computes its row block of
sigmoid(a_local[:, None] + b_full[None, :] + bias), stored as fp16. The host
concatenates the row blocks and upcasts to f32 (fp16 quantization of values
in (0,1) adds ~5e-4 max rel error vs the 2e-2 tolerance).

Device-side plan (per core), seg-major over four 2048-column segments:
  * pb[128, 2048] PSUM = b_full replicated across partitions, computed
    directly as wjrep^T @ x^T (K=128 matmuls; wjrep = w_j broadcast along
    the free dim).
  * The 8 row tiles of each segment are split across two engines, each tile
    ending in a 512KB fp16 store of [128, 2048]:
      - ACT tiles: sigmoid(pb + a_col) straight from PSUM, fp16 out.
      - DVE tiles: K=2 PE outer products build psum = 1 + e^-(a+c) * e^-b
        in [128, 512] pieces; DVE reciprocal gives sigmoid, fp16 out.
    This keeps both ACT (~36us) and DVE (~37us) under the DMA stream.
  * The auxiliary vectors come from tiny PE projections + transposes + ACT
    exp: a column (sigmoid bias), u = e^-(a+c) row (uo2), v = e^-b row (v2).

Queue/scheduling structure (what the cost model rewards):
  * All input loads + SBUF scatters on the SP HWDGE queue (650ns issue
    each, in critical order); all 32 output stores on the Pool SWDGE queue
    so the two streams never head-of-line block each other.
  * PE p-state warm-up matmuls so the first pb build runs at full clock.
  * Segment s+1's pb matmuls are emitted after segment s's outer products;
    chunk 1-3 v-row prep is merged into one exp + one scatter mid-segment-0.
  * Two value-identical-but-data-dependent operand replicas (wjrep4, wj3)
    force the list scheduler to order the v2-prep chain ahead of the first
    sigmoid and the xt2/xt3-gated projection matmuls behind the segment-0
    outer products — the scheduler's internal timing model would otherwise
    pick orders that stall the in-order ACT/PE/DVE queues.

The kernel is DMA-bound: 16.8MB fp16 stores + 2.4MB inputs per core at
360GB/s = 53.5us busy; simulated exec is ~61.6us (vs 112us for the f32
store baseline).
"""

import numpy as np

_N = 8192          # rows/cols of the output
_D = 128           # feature dim
_M = 8             # cores
_R = _N // _M      # 1024 rows per core
_RT = _R // 128    # 8 row tiles per core
_SEG = 2048        # output column segment width
_NSEG = _N // _SEG # 4 segments
_NCH = _N // _SEG  # proj chunks == segments (2048 cols each)

# (col_start, width) of each output column segment: narrow ones first so the
# store stream starts early, wide ones for steady state
_SEGS = ((0, 2048), (2048, 2048), (4096, 2048), (6144, 2048))
# per-segment row-tile assignment: number of ACT tiles (rest go to DVE)
_ACT_TILES = (5, 4, 4, 5)

_nc_cache = None


def _split_multi_waits(nc, mybir, max_keep=1):
    """Walrus on this toolchain only encodes ONE sem wait per instruction
    (NEURON_ISA_TPB_EVENTS has a single wait slot); Tile emits multi-wait
    sync_info. Split extras onto NoOps inserted right before the instruction
    on the same engine."""
    n_split = 0
    for fn in nc.m.functions:
        for bb in fn.blocks:
            newlist = []
            changed = False
            for inst in list(bb.instructions):
                si = inst.sync_info
                if si is not None and si.on_wait and len(si.on_wait) > max_keep:
                    waits = list(si.on_wait)
                    extra, keep = waits[:-max_keep], waits[-max_keep:]
                    for k, w in enumerate(extra):
                        newlist.append(
                            mybir.InstNoOp(
                                name=f"{inst.name}-waitsplit{k}",
                                engine=inst.engine,
                                sync_info=mybir.SyncInfo(on_wait=[w], on_update=[]),
                                bass_nofuse=True,
                            )
                        )
                        n_split += 1
                    inst.sync_info = mybir.SyncInfo(
                        on_wait=keep, on_update=list(si.on_update)
                    )
                    changed = True
                newlist.append(inst)
            if changed:
                bb.instructions = newlist
    return n_split


def _build():
    global _nc_cache
    if _nc_cache is not None:
        return _nc_cache

    import concourse.bass as bass
    import concourse.mybir as mybir
    from concourse.tile import TileContext

    f32 = mybir.dt.float32
    bf16 = mybir.dt.bfloat16
    fp16 = mybir.dt.float16
    Sigmoid = mybir.ActivationFunctionType.Sigmoid
    Identity = mybir.ActivationFunctionType.Identity
    Exp = mybir.ActivationFunctionType.Exp

    nc = bass.Bass("TRN2", debug=False, num_devices=_M)

    xt_d = nc.dram_tensor("xt", [_D, _N], bf16, kind="ExternalInput")    # full x^T
    # packed f32 consts: [:, :128] = eye(128), [:, 128] = linear bias b,
    # [:, 129] = w_j f32, [:, 130] = -b, [:, 131] = (w_i|w_j) bf16-pair
    # bitcast, [:, 132:644] = local x^T columns (bf16 pairs bitcast)
    cst_d = nc.dram_tensor("cst", [_D, _D + 4 + _R // 2], f32, kind="ExternalInput")
    ones_d = nc.dram_tensor("ones16", [1, _N], fp16, kind="ExternalInput")
    out_d = nc.dram_tensor("out", [_R, _N], fp16, kind="ExternalOutput")

    with TileContext(nc) as tc:
        with (
            tc.tile_pool(name="const", bufs=1) as cpool,
            tc.tile_pool(name="work", bufs=2) as wpool,
            tc.tile_pool(name="outp", bufs=6) as opool,
            tc.tile_pool(name="ppr", bufs=2, space="PSUM") as pr_pool,
            tc.tile_pool(name="pb", bufs=1, space="PSUM") as pb_pool,
            tc.tile_pool(name="po", bufs=2, space="PSUM") as po_pool,
        ):
            # All input loads + SBUF scatters go on the SP HWDGE queue (in
            # issue-critical order: ~650ns SEQ occupancy each); all output
            # stores go on the Pool SWDGE queue so the two streams never
            # head-of-line block each other.
            cst_sb = cpool.tile([128, _D + 4 + _R // 2], f32)
            nc.sync.dma_start(out=cst_sb[:], in_=cst_d[:])
            xt_sb = cpool.tile([128, _N], bf16)
            nc.sync.dma_start(out=xt_sb[:, 0:_SEGS[0][1]], in_=xt_d[:, 0:_SEGS[0][1]])
            for _s in range(1, len(_SEGS)):
                nc.sync.dma_start(
                    out=xt_sb[:, _SEGS[_s][0]:_SEGS[_s][0] + _SEGS[_s][1]],
                    in_=xt_d[:, _SEGS[_s][0]:_SEGS[_s][0] + _SEGS[_s][1]],
                )
            eye_sb = cst_sb[:, 0:_D]
            ccol_sb = cst_sb[:, _D:_D + 1]      # linear bias / partition
            wjcol_sb = cst_sb[:, _D + 1:_D + 2]  # w_j f32 / partition
            negc_sb = cst_sb[:, _D + 2:_D + 3]   # -bias / partition
            wij_sb = cst_sb[:, _D + 3:_D + 4].bitcast(bf16)   # [128, 2]
            xlt_sb = cst_sb[:, _D + 4:].bitcast(bf16)         # [128, 1024]
            v2 = cpool.tile([2, _N], fp16)      # row 0 = exp(-b), row 1 = ones
            nc.sync.dma_start(out=v2[1:2, :], in_=ones_d[:])
            # row 0 = exp(-(a_local+c)) for the 1024 local rows, row 1 = ones;
            # [2, 128] free-dim slices are K=2 lhsT operands (base partition 0)
            uo2 = cpool.tile([2, _R], fp16)
            nc.sync.dma_start(out=uo2[1:2, :], in_=ones_d[0:1, 0:_R])
            a_sb = cpool.tile([128, _RT], f32)

            # w_j replicated along the free dim: pb segments come straight
            # from wjrep^T @ xt (K=128), no b-row round-trip needed
            zer = wpool.tile([128, 128], bf16, tag="zer")
            nc.vector.memset(zer[:], 0.0)
            wjrep = cpool.tile([128, 128], bf16)
            nc.vector.tensor_scalar_add(out=wjrep[:], in0=zer[:], scalar1=wjcol_sb)

            def pb_build(s, last_lhsT=None):
                c0, w = _SEGS[s]
                pb = pb_pool.tile([128, w], f32, tag="pb", name=f"pb{s}")
                n_mm = w // 512
                for r in range(n_mm):
                    lhsT = wjrep
                    if last_lhsT is not None and r == n_mm - 1:
                        lhsT = last_lhsT
                    nc.tensor.matmul(
                        pb[:, 512 * r:512 * (r + 1)],
                        lhsT[:],
                        xt_sb[:, c0 + 512 * r:c0 + 512 * (r + 1)],
                    )
                return pb

            # PE p-state warm-up: dummy matmuls until real work arrives
            # (cold PE runs at 0.65GHz; ~3us of continuous work reaches 2.4GHz)
            pwarm = po_pool.tile([128, 512], f32, tag="po", name="pwarm")
            for w in range(9):
                nc.tensor.matmul(pwarm[:, 0:128], zer[:], zer[:],
                                 skip_group_check=True)

            # ---- local projection a = xl @ w_i ----
            psumL = pr_pool.tile([128, _RT], f32, tag="pr")
            for q in range(_RT):
                nc.tensor.matmul(
                    psumL[:, q:q + 1],
                    xlt_sb[:, 128 * q:128 * (q + 1)],
                    wij_sb[:, 0:1],
                )
            # u row via DVE copy + PE transpose (keeps ACT free so the exp
            # beats the first sigmoid into the engine), bias c folded into
            # the exp via the -c const column
            acol = wpool.tile([128, _RT], f32, tag="bc", name="acol")
            nc.vector.tensor_copy(out=acol[:], in_=psumL[:])
            aT = pr_pool.tile([_RT, 128], f32, tag="pr")
            nc.tensor.transpose(aT[:], acol[:], eye_sb)
            u8 = wpool.tile([_RT, 128], fp16, tag="u8")
            nc.scalar.activation(
                u8[:], aT[:], Exp, bias=negc_sb[0:_RT, :], scale=-1.0
            )
            nc.sync.dma_start(out=uo2[0:1, :], in_=u8[:])
            # a + c as per-partition sigmoid bias columns (off critical path)
            nc.scalar.activation(
                a_sb[:], psumL[:], Identity, bias=ccol_sb, scale=1.0
            )

            # ---- v2 row per column chunk (only the DVE tiles need it):
            # b column block -> transpose -> exp(-b) -> scatter into v2
            def proj_a(s):
                c0, w = _SEGS[s]
                nt = w // 128
                psumF = pr_pool.tile([128, nt], f32, tag="pr", name=f"psumF{s}")
                for t in range(nt):
                    tg = c0 // 128 + t
                    nc.tensor.matmul(
                        psumF[:, t:t + 1],
                        xt_sb[:, 128 * tg:128 * (tg + 1)],
                        wij_sb[:, 1:2],
                    )
                bcol = wpool.tile([128, nt], f32, tag="bc", name=f"bcol{s}")
                nc.vector.tensor_copy(out=bcol[:], in_=psumF[:])
                bT = pr_pool.tile([nt, 128], f32, tag="pr", name=f"bT{s}")
                nc.tensor.transpose(bT[:], bcol[:], eye_sb)
                return bT, bcol

            def proj_b(s, bT):
                nt = _SEGS[s][1] // 128
                vT16 = wpool.tile([nt, 128], fp16, tag="vt", name=f"vT16{s}")
                nc.scalar.activation(vT16[:], bT[:], Exp, bias=0.0, scale=-1.0)
                return vT16

            def proj_c(s, vT16):
                c0, w = _SEGS[s]
                nc.sync.dma_start(out=v2[0:1, c0:c0 + w], in_=vT16[:])

            bT0, bcol0 = proj_a(0)
            proj_c(0, proj_b(0, bT0))
            # w_j replica that is value-identical but data-dependent on
            # bcol0 (0*bcol0 + w_j). pb0's last matmul uses it, so sigmoid 0
            # cannot become ready before the bT0 -> vT16(0) -> v2 scatter
            # chain has started: the scheduler is forced to order the v2
            # prep ahead of the first sigmoid on ACT, which unblocks this
            # segment's DVE tiles ~2.5us earlier.
            wjrep4 = cpool.tile([128, 128], bf16)
            nc.vector.tensor_scalar(
                out=wjrep4[:], in0=zer[:], scalar1=bcol0[:, 0:1],
                scalar2=wjcol_sb, op0=mybir.AluOpType.mult,
                op1=mybir.AluOpType.add,
            )
            pb = pb_build(0, last_lhsT=wjrep4)


            # ---- seg-major main loop ----
            # Per segment: n_act row tiles on ACT (sigmoid straight from the
            # pb PSUM), the rest on DVE (reciprocal of 1 + u*v built by K=2
            # PE outer products), emitted in expected completion order so the
            # Pool store queue drains without head-of-line blocking.
            with nc.allow_low_precision("fp16 output tiles; tolerance 2e-2"):
                nseg = len(_SEGS)
                for s in range(nseg):
                    c0, w = _SEGS[s]
                    n_act = _ACT_TILES[s]
                    last = s + 1 >= nseg

                    def act_tile(q):
                        o = opool.tile([128, w], fp16, tag="o")
                        nc.scalar.activation(
                            o[:], pb[:], Sigmoid, bias=a_sb[:, q:q + 1],
                            scale=1.0,
                        )
                        nc.gpsimd.dma_start(
                            out=out_d[128 * q:128 * (q + 1), c0:c0 + w],
                            in_=o[:],
                        )

                    def dve_tile(q):
                        o = opool.tile([128, w], fp16, tag="o")
                        for j in range(w // 512):
                            po = po_pool.tile([128, 512], f32, tag="po")
                            nc.tensor.matmul(
                                po[:],
                                uo2[:, 128 * q:128 * (q + 1)],
                                v2[:, c0 + 512 * j:c0 + 512 * (j + 1)],
                            )
                            nc.vector.reciprocal(
                                out=o[:, 512 * j:512 * (j + 1)], in_=po[:]
                            )
                        nc.gpsimd.dma_start(
                            out=out_d[128 * q:128 * (q + 1), c0:c0 + w],
                            in_=o[:],
                        )

                    # interleave ACT and DVE tiles in expected completion
                    # order (per-tile: ACT ~0.83ns/col+init, DVE ~1.3ns/col)
                    t_act = w * 0.833e-3 + 0.19
                    t_dve = (w // 512) * 0.658 + 0.06
                    aq = list(range(n_act))
                    dq = list(range(n_act, _RT))
                    order = []
                    ta, td = 0.0, 0.0
                    while aq or dq:
                        if aq and (not dq or ta + t_act <= td + t_dve):
                            order.append((aq.pop(0), True))
                            ta += t_act
                        else:
                            order.append((dq.pop(0), False))
                            td += t_dve
                    for k, (q, on_act) in enumerate(order):
                        if on_act:
                            act_tile(q)
                        else:
                            dve_tile(q)
                        if s == 0 and k == 2:
                            # chunks 1-3 v-row prep, merged into one exp +
                            # one scatter. The wj copy is produced on ACT
                            # mid-segment, so the xt2/xt3-gated matmuls
                            # cannot be hoisted ahead of the seg-0 outer
                            # products (PE) or sigmoids (ACT) by the
                            # scheduler's optimistic internal timing.
                            wj3 = wpool.tile([128, 1], bf16, tag="wj3")
                            nc.scalar.activation(
                                wj3[:], o_gate[:, 0:1], Identity,
                                bias=wjcol_sb, scale=0.0,
                            )
                            psumM = pr_pool.tile([128, 48], f32, tag="pr",
                                                 name="psumM")
                            for t in range(48):
                                tg = 16 + t
                                nc.tensor.matmul(
                                    psumM[:, t:t + 1],
                                    xt_sb[:, 128 * tg:128 * (tg + 1)],
                                    wj3[:, 0:1],
                                )
                            bcolM = wpool.tile([128, 48], f32, tag="bc",
                                               name="bcolM")
                            # ACT copy, not DVE: a DVE copy would wait on
                            # psumM at the head of the in-order DVE queue and
                            # stall this segment's reciprocals behind it
                            nc.scalar.activation(
                                bcolM[:], psumM[:], Identity, bias=0.0,
                                scale=1.0,
                            )
                            bTM = pr_pool.tile([48, 128], f32, tag="pr",
                                               name="bTM")
                            nc.tensor.transpose(bTM[:], bcolM[:], eye_sb)
                            vTM = wpool.tile([48, 128], fp16, tag="vt",
                                             name="vTM")
                            nc.scalar.activation(vTM[:], bTM[:], Exp,
                                                 bias=0.0, scale=-1.0)
                            nc.sync.dma_start(
                                out=v2[0:1, 2048:8192], in_=vTM[:]
                            )

                    # next segment's pb matmuls: after this segment's outer
                    # products on PE (pb banks are free once ACT is done)
                    if not last:
                        pb = pb_build(s + 1)

    _split_multi_waits(nc, mybir)

    _nc_cache = nc
    return nc


_runner_cache = None


def _get_runner(nc):
    """Build (once) a jitted shard_map callable around the bass_exec custom
    call, so repeated kernel() calls skip the per-call retrace/recompile that
    run_bass_kernel_spmd's fresh closures would incur."""
    global _runner_cache
    if _runner_cache is not None:
        return _runner_cache

    import jax
    from jax.experimental.shard_map import shard_map
    from jax.sharding import Mesh, PartitionSpec
    from concourse import bass2jax
    import concourse.mybir as mybir

    bass2jax.install_neuronx_cc_hook()

    in_names, out_names, out_avals, zero_outs = [], [], [], []
    for alloc in nc.m.functions[0].allocations:
        if not isinstance(alloc, mybir.MemoryLocationSet):
            continue
        name = alloc.memorylocations[0].name
        if alloc.kind == "ExternalInput":
            in_names.append(name)
        elif alloc.kind == "ExternalOutput":
            out_names.append(name)
            shape = tuple(alloc.tensor_shape)
            dtype = mybir.dt.np(alloc.dtype)
            out_avals.append(jax.core.ShapedArray(shape, dtype))
            zero_outs.append(np.zeros(shape, dtype))

    partition_name = nc.partition_id_tensor.name if nc.partition_id_tensor else None
    if partition_name is not None:
        in_names = [n for n in in_names if n != partition_name]
    n_params = len(in_names)
    all_names = in_names + out_names
    if partition_name is not None:
        all_names = all_names + [partition_name]

    def _body(*args):
        operands = list(args)
        if partition_name is not None:
            operands.append(bass2jax.partition_id_tensor())
        outs = bass2jax._bass_exec_p.bind(
            *operands,
            out_avals=tuple(out_avals),
            in_names=tuple(all_names),
            out_names=tuple(out_names),
            lowering_input_output_aliases=(),
            sim_require_finite=True,
            sim_require_nnan=True,
            nc=nc,
        )
        return tuple(outs)

    devices = jax.devices()[:_M]
    mesh = Mesh(np.asarray(devices), ("core",))
    nspecs = n_params + len(out_names)
    fn = jax.jit(
        shard_map(
            _body,
            mesh=mesh,
            in_specs=(PartitionSpec("core"),) * nspecs,
            out_specs=(PartitionSpec("core"),) * len(out_names),
            check_rep=False,
        ),
        keep_unused=True,
    )
    # Stage the (all-zero) output operands on device once; without donation
    # they are never consumed, so every call reuses them instead of shipping
    # the zeros through the relay each time.
    from jax.sharding import NamedSharding

    sh = NamedSharding(mesh, PartitionSpec("core"))
    zeros_dev = [
        jax.device_put(np.zeros((_M * z.shape[0], *z.shape[1:]), z.dtype), sh)
        for z in zero_outs
    ]
    _runner_cache = (fn, in_names, zeros_dev)
    return _runner_cache


class _Res:
    exec_time_ns = None
    results = None
    mean_exec_time_ns = None
    instructions_and_trace = None


def _make_in_maps(inputs):
    import ml_dtypes

    x = np.ascontiguousarray(np.asarray(inputs["x"], dtype=np.float32))
    w = np.asarray(inputs["w"], dtype=np.float32)
    b = np.asarray(inputs["b"], dtype=np.float32)
    assert x.shape == (_N, _D), x.shape

    bf16 = ml_dtypes.bfloat16
    xT = np.ascontiguousarray(x.T.astype(bf16))          # [128, 8192]
    wij = np.ascontiguousarray(
        np.stack([w[0, :_D], w[0, _D:]], axis=1).astype(bf16)
    )                                                    # [128, 2]
    cst = np.zeros((_D, _D + 4 + _R // 2), dtype=np.float32)
    cst[:, :_D] = np.eye(_D, dtype=np.float32)
    cst[:, _D] = b[0]
    cst[:, _D + 1] = w[0, _D:]
    cst[:, _D + 2] = -b[0]
    cst[:, _D + 3] = wij.view(np.float32)[:, 0]

    ones16 = np.ones((1, _N), dtype=np.float16)

    in_maps = []
    for c in range(_M):
        cst_c = cst.copy()
        cst_c[:, _D + 4:] = np.ascontiguousarray(
            xT[:, c * _R:(c + 1) * _R]
        ).view(np.float32)
        in_maps.append({"xt": xT, "cst": cst_c, "ones16": ones16})
    return in_maps


def _run(inputs, trace=False, trace_cores=None):
    from concourse._compat import axon_active

    nc = _build()
    in_maps = _make_in_maps(inputs)

    if axon_active() and not trace:
        fn, in_names, zeros_dev = _get_runner(nc)
        args = [
            np.concatenate([m[name] for m in in_maps], axis=0) for name in in_names
        ] + list(zeros_dev)
        out_cat = np.asarray(fn(*args)[0])
        return _Res(), out_cat.reshape(_M * _R, _N).astype(np.float32)

    from concourse.bass_utils import run_bass_kernel_spmd

    res = run_bass_kernel_spmd(
        nc, in_maps, core_ids=list(range(_M)), trace=trace, trace_cores=trace_cores
    )
    out = np.concatenate([r["out"] for r in res.results], axis=0).astype(np.float32)
    return res, out


def kernel(**inputs):
    _, out = _run(inputs)
    return out
